# revision 9
# baseline (speedup 1.0000x reference)
"""Trainium2 Bass kernel for nn_Attention (dense_transformer).

Math (per fused-batch element, 32 total = b*m):
    qkv = x @ w_qkv ; split q,k,v into 8 heads of 64
    sim = (q/8) @ k^T  (+ pos_bias term that is constant along the softmax
                        axis -> provably no effect on softmax output, dropped)
    attn = softmax(sim); out = (attn @ v) heads-concat @ w_out
Sharding: pure data-parallel over the fused (b*m)=32 axis -> 4 elements
per core on 8 cores, no collectives. Weights replicated.

Kernel strategy (per core, all-transposed dataflow, bf16 matmuls):
    xT   = PE-transpose(x)                        [c, n]
    qT,kT (pair-stacked) = W_qk^T @ xT            [e_slice, n]  (psum f32)
    V    = xT-slices @ W_v   stored [n, h, 65] with a ones col per head
    S^T  = kT_h^T-slice @ qT_h  for a head PAIR concurrently (row-tiled
           64/64) into ONE wide 2-bank psum tile [128, 1024]
    P^T  = exp(0.125 * S^T)  one wide ACT op per pair-jt (no max
           subtraction: |logits| small), bf16 -> SBUF
    outT_h (rows 0..63) + L_h (row 64) = V1_h^T @ P^T  (ones-column trick)
    OT   = outT_h * gpsimd-broadcast(1/L_h)
    out  = OT-slices^T @ w_out  [n, c] -> chunked DMA out per 128-row tile

Pipeline: emission is woven at fine granularity: batch b's attention units
(ST pair-jt, PV head) interleave with batch b+1's transposes/projections and
batch b-1's output projection, pacing the PE stream against the ACT (exp)
drain rate so neither head-of-line blocks.
"""

import os
import sys

for _p in ("/root/.axon_site/_ro/trn_rl_repo", "/opt/trn_rl_repo"):
    if os.path.isdir(_p) and _p not in sys.path:
        sys.path.append(_p)

import numpy as np

# ---- problem constants (hardcoded per spec) ----
B, M, N, C = 4, 8, 512, 512
HEADS, DHEAD = 8, 64
E3 = 3 * 512
NCORES = 8
BPC = (B * M) // NCORES  # batch elements per core = 4

_cache = {}


def _build():
    import concourse.bass as bass
    import concourse.mybir as mybir
    import concourse.tile as tile
    from concourse import bacc
    from concourse.masks import make_identity

    f32 = mybir.dt.float32
    bf16 = mybir.dt.bfloat16
    EXP = mybir.ActivationFunctionType.Exp

    nc = bacc.Bacc("TRN2", target_bir_lowering=False, debug=False,
                   num_devices=NCORES)

    x_ext = nc.declare_dram_parameter("x", [BPC, N, C], f32, isOutput=False)
    wq_ext = nc.declare_dram_parameter("w_qkv", [C, E3], f32, isOutput=False)
    wo_ext = nc.declare_dram_parameter("w_out", [512, 512], f32, isOutput=False)
    out_ext = nc.declare_dram_parameter("out", [BPC, N, C], f32, isOutput=True)

    from contextlib import ExitStack

    with tile.TileContext(nc) as tc, ExitStack() as ctx:
        # ---------------- pools ----------------
        p_const = ctx.enter_context(tc.tile_pool(name="const", bufs=1))
        p_x = ctx.enter_context(tc.tile_pool(name="x", bufs=2))
        p_xT = ctx.enter_context(tc.tile_pool(name="xT", bufs=2))
        p_qk = ctx.enter_context(tc.tile_pool(name="qk", bufs=2))
        p_v = ctx.enter_context(tc.tile_pool(name="v", bufs=2))
        p_pt = ctx.enter_context(tc.tile_pool(name="pt", bufs=2))
        p_oT = ctx.enter_context(tc.tile_pool(name="oT", bufs=2))
        p_out = ctx.enter_context(tc.tile_pool(name="out", bufs=2))
        p_small = ctx.enter_context(tc.tile_pool(name="small", bufs=4))

        # PSUM budget is exactly 8 banks, statically reserved per tag:
        #   st   2 x [128,512] f32 (1 bank each; bisect-revert) -> 2
        #   tr   1 x [128,2,512] bf16 (two ct chunks)     -> 1
        #   pj   3 x [128,512]  f32, shared by the qkv/out projection
        #        groups AND the PV accumulators           -> 3
        ps_st = ctx.enter_context(tc.tile_pool(name="ps_st", bufs=2, space="PSUM"))
        ps_tr = ctx.enter_context(tc.tile_pool(name="ps_tr", bufs=1, space="PSUM"))
        ps_pj = ctx.enter_context(tc.tile_pool(name="ps_pj", bufs=3, space="PSUM"))

        # ---------------- constants + initial DMAs ----------------
        # Emission order at startup: identity (tiny, needed by batch-0
        # transposes), then batch-0's x chunks on the (idle) sync HWDGE
        # queue, then weights on the gpsimd SWDGE queues.
        ident = p_const.tile([128, 128], bf16)
        make_identity(nc, ident[:])

        # batch-0 x: four independent column-chunk DMAs (f32, sync queue)
        # so cast+transpose of chunk ct can start the moment it lands.
        x0f = []
        x0c = []
        x_r0 = x_ext[0].rearrange("(nt p) c -> p nt c", p=128)
        for ct in range(4):
            t = p_const.tile([128, 4, 128], f32, tag=f"x0f{ct}", name=f"x0f{ct}")
            nc.sync.dma_start(out=t[:], in_=x_r0[:, :, ct * 128:(ct + 1) * 128])
            x0f.append(t)
            x0c.append(p_const.tile([128, 4, 128], bf16, tag=f"x0c{ct}",
                                    name=f"x0c{ct}"))

        # weights: gpsimd SWDGE cast-DMAs straight to bf16. Per-slice tiles
        # for the q/k stationary slices so each projection waits only on its
        # own slice's DMA.
        wq_r = wq_ext.ap().rearrange("(ct p) e -> p ct e", p=128)
        wqk_t = []
        for s in range(8):
            t = p_const.tile([128, 4, 128], bf16, tag=f"wq{s}", name=f"wq{s}")
            nc.gpsimd.dma_start(out=t[:], in_=wq_r[:, :, s * 128:(s + 1) * 128])
            wqk_t.append(t)
        wv_sb = p_const.tile([128, 4, 512], bf16)
        nc.gpsimd.dma_start(out=wv_sb[:], in_=wq_r[:, :, 1024:1536])
        wo_sb = p_const.tile([128, 4, 512], bf16)
        nc.gpsimd.dma_start(
            out=wo_sb[:],
            in_=wo_ext.ap().rearrange("(t p) c -> p t c", p=128))

        # batch-0 casts f32 -> bf16 (DVE), one per chunk
        for ct in range(4):
            nc.vector.tensor_copy(x0c[ct][:], x0f[ct][:])

        # ---------------- per-batch stage emitters ----------------
        def stage_x(b):
            """x [512,512] f32 -> SBUF bf16 via SWDGE cast DMA (b>0)."""
            x_sb = p_x.tile([128, 4, C], bf16, tag="x", name="x_sb")
            nc.gpsimd.dma_start(
                out=x_sb[:],
                in_=x_ext[b].rearrange("(nt p) c -> p nt c", p=128))
            return x_sb

        def stage_prep(b, x_sb):
            """Return (qkT, v_sb, [unit thunks]) for batch b's transposes +
            projections. x_sb is None for b==0 (per-chunk tiles)."""
            xT = p_xT.tile([128, 4, N], bf16, tag="xT", name="xT")
            qkT = p_qk.tile([128, 8, N], bf16, tag="qkT", name="qkT")
            v_sb = p_v.tile([128, 4, 8, 65], bf16, tag="v", name="v_sb")
            thunks = []

            def x_ap(ct, nt):
                if x_sb is None:
                    return x0c[ct][:, nt, :]
                return x_sb[:, nt, ct * 128:(ct + 1) * 128]

            def tr2(cp):
                # two ct chunks per unit -> one 1-bank psum tile + one wide
                # DVE copy
                tr_ps = ps_tr.tile([128, 2, 512], bf16, tag="tr", name="tr_ps")
                for i in range(2):
                    ct = 2 * cp + i
                    for nt in range(4):
                        nc.tensor.transpose(
                            tr_ps[:, i, nt * 128:(nt + 1) * 128],
                            x_ap(ct, nt), ident[:])
                nc.vector.tensor_copy(xT[:, 2 * cp:2 * cp + 2, :], tr_ps[:])

            def v_ones():
                nc.gpsimd.memset(v_sb[:, :, :, 64:65], 1.0)

            def proj_qk(s):
                pr_ps = ps_pj.tile([128, N], f32, tag="pj", name="pr_ps")
                for ct in range(4):
                    nc.tensor.matmul(
                        pr_ps[:], wqk_t[s][:, ct, :], xT[:, ct, :],
                        start=(ct == 0), stop=(ct == 3))
                # alternate psum->sbuf drain between ACT and DVE
                if s % 2 == 0:
                    nc.scalar.copy(qkT[:, s, :], pr_ps[:])
                else:
                    nc.vector.tensor_copy(qkT[:, s, :], pr_ps[:])

            def proj_v(nt):
                pv_ps = ps_pj.tile([128, N], f32, tag="pj", name="pv_ps")
                for ct in range(4):
                    nc.tensor.matmul(
                        pv_ps[:],
                        xT[:, ct, nt * 128:(nt + 1) * 128],
                        wv_sb[:, ct, :],
                        start=(ct == 0), stop=(ct == 3))
                nc.vector.tensor_copy(
                    v_sb[:, nt, :, 0:64],
                    pv_ps[:].rearrange("p (h d) -> p h d", d=64))

            for cp in range(2):
                thunks.append(lambda cp=cp: tr2(cp))
            thunks.append(v_ones)
            for s in range(8):
                thunks.append(lambda s=s: proj_qk(s))
            for nt in range(4):
                thunks.append(lambda nt=nt: proj_v(nt))
            return qkT, v_sb, thunks

        def stage_attn_units(qkT, v_sb):
            """Return (oT, [24 unit thunks]) for one batch's attention.
            Unit order keeps S^T one head-pair ahead of PV."""
            oT = p_oT.tile([128, 4, N], bf16, tag="oT", name="oT")
            pts = {}

            def st_unit(pair, jt):
                if jt == 0:
                    pts[pair] = p_pt.tile([128, 4, 1024], bf16, tag="pt",
                                          name="pt")
                for sub in range(2):
                    lo, hi = sub * 64, (sub + 1) * 64
                    st_ps = ps_st.tile([128, N], f32, tag="st", name="st_ps")
                    nc.tensor.matmul(
                        st_ps[:],
                        qkT[lo:hi, 4 + pair, jt * 128:(jt + 1) * 128],
                        qkT[lo:hi, pair, :],
                        start=True, stop=True)
                    nc.scalar.activation(
                        pts[pair][:, jt, sub * 512:(sub + 1) * 512],
                        st_ps[:], EXP, scale=float(DHEAD) ** -0.5)

            def pv_unit(pair, sub):
                pt = pts[pair]
                h = 2 * pair + sub
                ot_ps = ps_pj.tile([128, N], f32, tag="pj", name="ot_ps")
                for jt in range(4):
                    nc.tensor.matmul(
                        ot_ps[0:65, :],
                        v_sb[:, jt, h, :],
                        pt[:, jt, sub * 512:(sub + 1) * 512],
                        start=(jt == 0), stop=(jt == 3))
                lrow = p_small.tile([1, N], f32, tag="lrow", name="lrow")
                nc.vector.tensor_copy(lrow[:], ot_ps[64:65, :])
                invl = p_small.tile([1, N], f32, tag="invl", name="invl")
                nc.vector.reciprocal_approx_fast(invl[:], lrow[:])
                bc_sb = p_small.tile([64, N], f32, tag="bc_sb", name="bc_sb")
                nc.gpsimd.partition_broadcast(bc_sb[:], invl[:])
                nc.vector.tensor_mul(
                    oT[sub * 64:(sub + 1) * 64, pair, :],
                    ot_ps[0:64, :], bc_sb[:])

            units = []
            order = (
                [(0, jt) for jt in range(4)] + [(1, jt) for jt in range(4)]
                + ["pv00", "pv01"] + [(2, jt) for jt in range(4)]
                + ["pv10", "pv11"] + [(3, jt) for jt in range(4)]
                + ["pv20", "pv21", "pv30", "pv31"])
            for u in order:
                if isinstance(u, tuple):
                    units.append(lambda p=u[0], j=u[1]: st_unit(p, j))
                else:
                    units.append(
                        lambda p=int(u[2]), s=int(u[3]): pv_unit(p, s))
            return oT, units

        def stage_out_units(b, oT):
            """4 unit thunks: out-projection + copy + chunked DMA per
            128-row tile of the output."""
            out_sb = p_out.tile([128, 4, C], f32, tag="out", name="out_sb")
            out_r = out_ext[b].rearrange("(nt p) c -> p nt c", p=128)

            def unit(nt):
                f_ps = ps_pj.tile([128, C], f32, tag="pj", name="f_ps")
                for t in range(4):
                    nc.tensor.matmul(
                        f_ps[:],
                        oT[:, t, nt * 128:(nt + 1) * 128],
                        wo_sb[:, t, :],
                        start=(t == 0), stop=(t == 3))
                if nt % 2 == 0:
                    nc.scalar.copy(out_sb[:, nt, :], f_ps[:])
                else:
                    nc.vector.tensor_copy(out_sb[:, nt, :], f_ps[:])
                nc.sync.dma_start(out=out_r[:, nt, :], in_=out_sb[:, nt, :])

            return [lambda nt=nt: unit(nt) for nt in range(4)]

        # ---------------- cross-batch woven pipeline ----------------
        qkT, v_sb, prep_thunks = stage_prep(0, None)
        for t in prep_thunks:
            t()
        out_units = []  # deferred out-projection units of batch b-1
        for b in range(BPC):
            oT, attn_units = stage_attn_units(qkT, v_sb)
            filler = list(out_units)
            if b + 1 < BPC:
                x_next = stage_x(b + 1)
                qkT_n, v_n, prep_n = stage_prep(b + 1, x_next)
                filler += prep_n
                qkT_next, v_next = qkT_n, v_n
            # weave: alternate attention unit / filler unit
            na, nf = len(attn_units), len(filler)
            fi = 0
            for ai, au in enumerate(attn_units):
                au()
                want = (ai + 1) * nf // na
                while fi < want:
                    filler[fi]()
                    fi += 1
            while fi < nf:
                filler[fi]()
                fi += 1
            out_units = stage_out_units(b, oT)
            if b + 1 < BPC:
                qkT, v_sb = qkT_next, v_next
        for u in out_units:
            u()

    nc.compile()
    return nc


def _get_nc():
    if "nc" not in _cache:
        _cache["nc"] = _build()
    return _cache["nc"]


def kernel(x, pos_bias=None, w_qkv=None, w_out=None, **_ignored):
    from concourse.bass_utils import run_bass_kernel_spmd

    nc = _get_nc()
    xf = np.ascontiguousarray(np.asarray(x, dtype=np.float32).reshape(B * M, N, C))
    wq = np.ascontiguousarray(np.asarray(w_qkv, dtype=np.float32))
    wo = np.ascontiguousarray(np.asarray(w_out, dtype=np.float32))
    in_maps = [
        {"x": xf[i * BPC:(i + 1) * BPC], "w_qkv": wq, "w_out": wo}
        for i in range(NCORES)
    ]
    res = run_bass_kernel_spmd(
        nc, in_maps, core_ids=list(range(NCORES)),
        trace=bool(_cache.get("trace", False)))
    _cache["last_result"] = res
    out = np.concatenate([res.results[i]["out"] for i in range(NCORES)], axis=0)
    return out.reshape(B, M, N, C).astype(np.float32)


# revision 11
# speedup vs baseline: 1.0735x; 1.0735x over previous
"""Trainium2 Bass kernel for nn_Attention (dense_transformer).

Math (per fused-batch element, 32 total = b*m):
    qkv = x @ w_qkv ; split q,k,v into 8 heads of 64
    sim = (q/8) @ k^T  (+ pos_bias term that is constant along the softmax
                        axis -> provably no effect on softmax output, dropped)
    attn = softmax(sim); out = (attn @ v) heads-concat @ w_out
Sharding: pure data-parallel over the fused (b*m)=32 axis -> 4 elements
per core on 8 cores, no collectives. Weights replicated.

Kernel strategy (per core, all-transposed dataflow, bf16 matmuls):
    xT   = PE-transpose(x)                        [c, n]
    qT,kT (pair-stacked) = W_qk^T @ xT            [e_slice, n]  (psum f32)
    V    = xT-slices @ W_v   stored [n, h, 65] with a ones col per head
    S^T  = kT_h^T-slice @ qT_h  for a head PAIR concurrently (row-tiled
           64/64) into ONE wide 2-bank psum tile [128, 1024]
    P^T  = exp(0.125 * S^T)  one wide ACT op per pair-jt (no max
           subtraction: |logits| small), bf16 -> SBUF
    outT_h (rows 0..63) + L_h (row 64) = V1_h^T @ P^T  (ones-column trick)
    OT   = outT_h * gpsimd-broadcast(1/L_h)
    out  = OT-slices^T @ w_out  [n, c] -> chunked DMA out per 128-row tile

Pipeline: emission is woven at fine granularity: batch b's attention units
(ST pair-jt, PV head) interleave with batch b+1's transposes/projections and
batch b-1's output projection, pacing the PE stream against the ACT (exp)
drain rate so neither head-of-line blocks.
"""

import os
import sys

for _p in ("/root/.axon_site/_ro/trn_rl_repo", "/opt/trn_rl_repo"):
    if os.path.isdir(_p) and _p not in sys.path:
        sys.path.append(_p)

import numpy as np

# ---- problem constants (hardcoded per spec) ----
B, M, N, C = 4, 8, 512, 512
HEADS, DHEAD = 8, 64
E3 = 3 * 512
NCORES = 8
BPC = (B * M) // NCORES  # batch elements per core = 4

_cache = {}


def _build():
    import concourse.bass as bass
    import concourse.mybir as mybir
    import concourse.tile as tile
    from concourse import bacc
    from concourse.masks import make_identity

    f32 = mybir.dt.float32
    bf16 = mybir.dt.bfloat16
    EXP = mybir.ActivationFunctionType.Exp

    nc = bacc.Bacc("TRN2", target_bir_lowering=False, debug=False,
                   num_devices=NCORES)

    x_ext = nc.declare_dram_parameter("x", [BPC, N, C], f32, isOutput=False)
    wq_ext = nc.declare_dram_parameter("w_qkv", [C, E3], f32, isOutput=False)
    wo_ext = nc.declare_dram_parameter("w_out", [512, 512], f32, isOutput=False)
    # output leaves the device as bf16 (host casts back to f32): halves the
    # tail DMA and stays well inside the 2e-2 tolerance
    out_ext = nc.declare_dram_parameter("out", [BPC, N, C], bf16, isOutput=True)

    from contextlib import ExitStack

    with tile.TileContext(nc) as tc, ExitStack() as ctx:
        # ---------------- pools ----------------
        p_const = ctx.enter_context(tc.tile_pool(name="const", bufs=1))
        p_x = ctx.enter_context(tc.tile_pool(name="x", bufs=2))
        p_xT = ctx.enter_context(tc.tile_pool(name="xT", bufs=2))
        p_qk = ctx.enter_context(tc.tile_pool(name="qk", bufs=2))
        p_v = ctx.enter_context(tc.tile_pool(name="v", bufs=2))
        p_pt = ctx.enter_context(tc.tile_pool(name="pt", bufs=2))
        p_oT = ctx.enter_context(tc.tile_pool(name="oT", bufs=2))
        p_out = ctx.enter_context(tc.tile_pool(name="out", bufs=2))
        p_small = ctx.enter_context(tc.tile_pool(name="small", bufs=4))

        # PSUM budget is exactly 8 banks, statically reserved per tag:
        #   st   2 x [128,512] f32 (1 bank each; bisect-revert) -> 2
        #   tr   1 x [128,2,512] bf16 (two ct chunks)     -> 1
        #   pj   3 x [128,512]  f32, shared by the qkv/out projection
        #        groups AND the PV accumulators           -> 3
        ps_st = ctx.enter_context(tc.tile_pool(name="ps_st", bufs=2, space="PSUM"))
        ps_tr = ctx.enter_context(tc.tile_pool(name="ps_tr", bufs=1, space="PSUM"))
        ps_pj = ctx.enter_context(tc.tile_pool(name="ps_pj", bufs=3, space="PSUM"))

        # ---------------- constants + initial DMAs ----------------
        # Emission order at startup: identity (tiny, needed by batch-0
        # transposes), then batch-0's x chunks on the (idle) sync HWDGE
        # queue, then weights on the gpsimd SWDGE queues.
        ident = p_const.tile([128, 128], bf16)
        make_identity(nc, ident[:])

        # batch-0 x: four independent column-chunk DMAs (f32, sync queue)
        # so cast+transpose of chunk ct can start the moment it lands.
        x0f = []
        x0c = []
        x_r0 = x_ext[0].rearrange("(nt p) c -> p nt c", p=128)
        for ct in range(4):
            t = p_const.tile([128, 4, 128], f32, tag=f"x0f{ct}", name=f"x0f{ct}")
            nc.sync.dma_start(out=t[:], in_=x_r0[:, :, ct * 128:(ct + 1) * 128])
            x0f.append(t)
            x0c.append(p_const.tile([128, 4, 128], bf16, tag=f"x0c{ct}",
                                    name=f"x0c{ct}"))

        # weights: gpsimd SWDGE cast-DMAs straight to bf16. Per-slice tiles
        # for the q/k stationary slices so each projection waits only on its
        # own slice's DMA.
        wq_r = wq_ext.ap().rearrange("(ct p) e -> p ct e", p=128)
        wqk_t = []
        for s in range(8):
            t = p_const.tile([128, 4, 128], bf16, tag=f"wq{s}", name=f"wq{s}")
            nc.gpsimd.dma_start(out=t[:], in_=wq_r[:, :, s * 128:(s + 1) * 128])
            wqk_t.append(t)
        wv_sb = p_const.tile([128, 4, 512], bf16)
        nc.gpsimd.dma_start(out=wv_sb[:], in_=wq_r[:, :, 1024:1536])
        wo_sb = p_const.tile([128, 4, 512], bf16)
        nc.gpsimd.dma_start(
            out=wo_sb[:],
            in_=wo_ext.ap().rearrange("(t p) c -> p t c", p=128))

        # batch-0 casts f32 -> bf16 (DVE), one per chunk
        for ct in range(4):
            nc.vector.tensor_copy(x0c[ct][:], x0f[ct][:])

        # ---------------- per-batch stage emitters ----------------
        def stage_x(b):
            """x [512,512] f32 -> SBUF bf16 via SWDGE cast DMA (b>0)."""
            x_sb = p_x.tile([128, 4, C], bf16, tag="x", name="x_sb")
            nc.gpsimd.dma_start(
                out=x_sb[:],
                in_=x_ext[b].rearrange("(nt p) c -> p nt c", p=128))
            return x_sb

        def stage_prep(b, x_sb):
            """Return (qkT, v_sb, [unit thunks]) for batch b's transposes +
            projections. x_sb is None for b==0 (per-chunk tiles)."""
            xT = p_xT.tile([128, 4, N], bf16, tag="xT", name="xT")
            qkT = p_qk.tile([128, 8, N], bf16, tag="qkT", name="qkT")
            v_sb = p_v.tile([128, 4, 8, 65], bf16, tag="v", name="v_sb")
            thunks = []

            def x_ap(ct, nt):
                if x_sb is None:
                    return x0c[ct][:, nt, :]
                return x_sb[:, nt, ct * 128:(ct + 1) * 128]

            def tr2(cp):
                # two ct chunks per unit -> one 1-bank psum tile + one wide
                # DVE copy
                tr_ps = ps_tr.tile([128, 2, 512], bf16, tag="tr", name="tr_ps")
                for i in range(2):
                    ct = 2 * cp + i
                    for nt in range(4):
                        nc.tensor.transpose(
                            tr_ps[:, i, nt * 128:(nt + 1) * 128],
                            x_ap(ct, nt), ident[:])
                nc.vector.tensor_copy(xT[:, 2 * cp:2 * cp + 2, :], tr_ps[:])

            def v_ones():
                nc.gpsimd.memset(v_sb[:, :, :, 64:65], 1.0)

            def proj_qk(s):
                pr_ps = ps_pj.tile([128, N], f32, tag="pj", name="pr_ps")
                for ct in range(4):
                    nc.tensor.matmul(
                        pr_ps[:], wqk_t[s][:, ct, :], xT[:, ct, :],
                        start=(ct == 0), stop=(ct == 3))
                # alternate psum->sbuf drain between ACT and DVE
                if s % 2 == 0:
                    nc.scalar.copy(qkT[:, s, :], pr_ps[:])
                else:
                    nc.vector.tensor_copy(qkT[:, s, :], pr_ps[:])

            def proj_v(nt):
                pv_ps = ps_pj.tile([128, N], f32, tag="pj", name="pv_ps")
                for ct in range(4):
                    nc.tensor.matmul(
                        pv_ps[:],
                        xT[:, ct, nt * 128:(nt + 1) * 128],
                        wv_sb[:, ct, :],
                        start=(ct == 0), stop=(ct == 3))
                nc.vector.tensor_copy(
                    v_sb[:, nt, :, 0:64],
                    pv_ps[:].rearrange("p (h d) -> p h d", d=64))

            for cp in range(2):
                thunks.append(lambda cp=cp: tr2(cp))
            thunks.append(v_ones)
            for s in range(8):
                thunks.append(lambda s=s: proj_qk(s))
            for nt in range(4):
                thunks.append(lambda nt=nt: proj_v(nt))
            return qkT, v_sb, thunks

        def stage_attn_units(qkT, v_sb):
            """Return (oT, [24 unit thunks]) for one batch's attention.
            Unit order keeps S^T one head-pair ahead of PV."""
            oT = p_oT.tile([128, 4, N], bf16, tag="oT", name="oT")
            pts = {}

            def st_unit(pair, jt):
                if jt == 0:
                    pts[pair] = p_pt.tile([128, 4, 1024], bf16, tag="pt",
                                          name="pt")
                wide = ps_st.tile([128, 1024], f32, tag="st", name="st_ps")
                for sub in range(2):
                    lo, hi = sub * 64, (sub + 1) * 64
                    nc.tensor.matmul(
                        wide[:, sub * 512:(sub + 1) * 512],
                        qkT[lo:hi, 4 + pair, jt * 128:(jt + 1) * 128],
                        qkT[lo:hi, pair, :],
                        start=True, stop=True)
                nc.scalar.activation(
                    pts[pair][:, jt, :], wide[:], EXP,
                    scale=float(DHEAD) ** -0.5)

            def pv_unit(pair, sub):
                pt = pts[pair]
                h = 2 * pair + sub
                ot_ps = ps_pj.tile([128, N], f32, tag="pj", name="ot_ps")
                for jt in range(4):
                    nc.tensor.matmul(
                        ot_ps[0:65, :],
                        v_sb[:, jt, h, :],
                        pt[:, jt, sub * 512:(sub + 1) * 512],
                        start=(jt == 0), stop=(jt == 3))
                lrow = p_small.tile([1, N], f32, tag="lrow", name="lrow")
                nc.vector.tensor_copy(lrow[:], ot_ps[64:65, :])
                invl = p_small.tile([1, N], f32, tag="invl", name="invl")
                nc.vector.reciprocal_approx_fast(invl[:], lrow[:])
                bc_sb = p_small.tile([64, N], f32, tag="bc_sb", name="bc_sb")
                nc.gpsimd.partition_broadcast(bc_sb[:], invl[:])
                nc.vector.tensor_mul(
                    oT[sub * 64:(sub + 1) * 64, pair, :],
                    ot_ps[0:64, :], bc_sb[:])

            units = []
            order = (
                [(0, jt) for jt in range(4)] + [(1, jt) for jt in range(4)]
                + ["pv00", "pv01"] + [(2, jt) for jt in range(4)]
                + ["pv10", "pv11"] + [(3, jt) for jt in range(4)]
                + ["pv20", "pv21", "pv30", "pv31"])
            for u in order:
                if isinstance(u, tuple):
                    units.append(lambda p=u[0], j=u[1]: st_unit(p, j))
                else:
                    units.append(
                        lambda p=int(u[2]), s=int(u[3]): pv_unit(p, s))
            return oT, units

        def stage_out_units(b, oT):
            """4 unit thunks: out-projection + copy + chunked DMA per
            128-row tile of the output."""
            out_sb = p_out.tile([128, 4, C], bf16, tag="out", name="out_sb")
            out_r = out_ext[b].rearrange("(nt p) c -> p nt c", p=128)

            def unit(nt):
                f_ps = ps_pj.tile([128, C], f32, tag="pj", name="f_ps")
                for t in range(4):
                    nc.tensor.matmul(
                        f_ps[:],
                        oT[:, t, nt * 128:(nt + 1) * 128],
                        wo_sb[:, t, :],
                        start=(t == 0), stop=(t == 3))
                if nt % 2 == 0:
                    nc.scalar.copy(out_sb[:, nt, :], f_ps[:])
                else:
                    nc.vector.tensor_copy(out_sb[:, nt, :], f_ps[:])
                nc.sync.dma_start(out=out_r[:, nt, :], in_=out_sb[:, nt, :])

            return [lambda nt=nt: unit(nt) for nt in range(4)]

        # ---------------- cross-batch woven pipeline ----------------
        qkT, v_sb, prep_thunks = stage_prep(0, None)
        for t in prep_thunks:
            t()
        out_units = []  # deferred out-projection units of batch b-1
        for b in range(BPC):
            oT, attn_units = stage_attn_units(qkT, v_sb)
            filler = list(out_units)
            if b + 1 < BPC:
                x_next = stage_x(b + 1)
                qkT_n, v_n, prep_n = stage_prep(b + 1, x_next)
                filler += prep_n
                qkT_next, v_next = qkT_n, v_n
            # weave: alternate attention unit / filler unit
            na, nf = len(attn_units), len(filler)
            fi = 0
            for ai, au in enumerate(attn_units):
                au()
                want = (ai + 1) * nf // na
                while fi < want:
                    filler[fi]()
                    fi += 1
            while fi < nf:
                filler[fi]()
                fi += 1
            out_units = stage_out_units(b, oT)
            if b + 1 < BPC:
                qkT, v_sb = qkT_next, v_next
        for u in out_units:
            u()

    nc.compile()
    return nc


def _get_nc():
    if "nc" not in _cache:
        _cache["nc"] = _build()
    return _cache["nc"]


def kernel(x, pos_bias=None, w_qkv=None, w_out=None, **_ignored):
    from concourse.bass_utils import run_bass_kernel_spmd

    nc = _get_nc()
    xf = np.ascontiguousarray(np.asarray(x, dtype=np.float32).reshape(B * M, N, C))
    wq = np.ascontiguousarray(np.asarray(w_qkv, dtype=np.float32))
    wo = np.ascontiguousarray(np.asarray(w_out, dtype=np.float32))
    in_maps = [
        {"x": xf[i * BPC:(i + 1) * BPC], "w_qkv": wq, "w_out": wo}
        for i in range(NCORES)
    ]
    res = run_bass_kernel_spmd(
        nc, in_maps, core_ids=list(range(NCORES)),
        trace=bool(_cache.get("trace", False)))
    _cache["last_result"] = res
    out = np.concatenate(
        [np.asarray(res.results[i]["out"]) for i in range(NCORES)], axis=0)
    return out.reshape(B, M, N, C).astype(np.float32)
